# revision 8
# baseline (speedup 1.0000x reference)
"""Causal self-attention (B=2, S=2048, D=1024, H=16) on 8 trn2 NeuronCores.

Sharding: core c -> batch b = c//4, head-group hg = c%4 (4 heads/core).
Each core computes qkv projection for its heads, causal attention, and a
partial output projection (rows hg*256:(hg+1)*256 of w_proj). Host sums the
4 partials per batch.

Device-side orientation is fully transposed so no PE transposes are needed:
  qkT  [512, 2048]  = (x @ w_qk)^T   (rows 0-255 = qT for 4 heads, 256-511 = kT)
  v    [2048, 260]  = x @ w_v, augmented with a ones column per head
                      (65th column -> softmax denominator comes out of the
                       context matmul for free)
  scoresT [Sk, Sq]  = kT_tile^T-matmul, softmax denom handled via the ones row
  ctxT [65, Sq]     accumulated over causal Sk chunks in PSUM
  normalization     = reciprocal of row 64, broadcast via a K=1 matmul,
                      one DVE multiply per head-pair
  proj partial      = ctx^T used directly as lhsT (no transpose needed)
"""

import os

import numpy as np

B, S, D, H = 2, 2048, 1024, 16
HD = D // H  # 64
N_CORES = 8
NHC = 4  # heads per core
QB = 512  # query block (free dim of attention matmuls)
NQB = S // QB  # 4
NST = S // 128  # 16 s-tiles
NDC = D // 128  # 8 contraction chunks

MODE = os.environ.get("TRN_ATTN_MODE", "mixed")  # "f32" | "bf16" | "mixed"
XPART = os.environ.get("TRN_ATTN_XPART", "1") == "1"

_CACHE = {}


def _build(mode, xpart):
    import concourse.bass as bass
    import concourse.tile as tile
    from concourse import bacc, mybir

    f32 = mybir.dt.float32
    f32r = mybir.dt.float32r
    bf16 = mybir.dt.bfloat16
    EXP = mybir.ActivationFunctionType.Exp

    # pdt: qkv-projection inputs (DMA-fed, so f32r is legal); adt: attention
    # tensors (compute-produced); cdt: ctx/proj tensors (compute-produced)
    if mode == "mixed":
        pdt, adt, cdt = f32r, bf16, bf16
    elif mode == "bf16":
        pdt, adt, cdt = bf16, bf16, bf16
    else:
        pdt, adt, cdt = f32, f32, f32

    nc = bacc.Bacc("TRN2", target_bir_lowering=False, debug=False,
                   num_devices=N_CORES)

    np_adt = mybir.dt.np(adt)

    xt_d = nc.dram_tensor("xt", [D, S], pdt, kind="ExternalInput").ap()
    wqk_d = nc.dram_tensor("wqk", [D, 512], pdt, kind="ExternalInput").ap()
    wv_d = nc.dram_tensor("wv", [D, 256], pdt, kind="ExternalInput").ap()
    wp_d = nc.dram_tensor("wp", [256, D], cdt, kind="ExternalInput").ap()
    bqk_d = nc.dram_tensor("bqk", [128, 4], f32, kind="ExternalInput").ap()
    tri_d = nc.dram_tensor("tri", [128, 128], adt, kind="ExternalInput").ap()
    out_d = nc.dram_tensor("out", [S, D], f32, kind="ExternalOutput").ap()

    from contextlib import ExitStack
    with tile.TileContext(nc) as tc, ExitStack() as ctx:
        pool = lambda name, bufs: ctx.enter_context(tc.tile_pool(name=name, bufs=bufs))
        ppool = lambda name, bufs: ctx.enter_context(
            tc.tile_pool(name=name, bufs=bufs, space="PSUM"))

        stat = pool("stat", 1)

        # ---- static sbuf tensors + loads ----
        xt = []
        for d in range(NDC):
            t = stat.tile([128, S], pdt, tag=f"xt{d}")
            nc.sync.dma_start(t[:], xt_d[d * 128:(d + 1) * 128, :])
            xt.append(t)
        wqk = []
        for d in range(NDC):
            t = stat.tile([128, 512], pdt, tag=f"wqk{d}")
            nc.sync.dma_start(t[:], wqk_d[d * 128:(d + 1) * 128, :])
            wqk.append(t)
        wv = []
        for d in range(NDC):
            t = stat.tile([128, 256], pdt, tag=f"wv{d}")
            nc.sync.dma_start(t[:], wv_d[d * 128:(d + 1) * 128, :])
            wv.append(t)
        if xpart:
            wp = []
            for p in range(2):
                t = stat.tile([128, D], cdt, tag=f"wp{p}")
                nc.sync.dma_start(t[:], wp_d[p * 128:(p + 1) * 128, :])
                wp.append(t)
        else:
            wp = []
            for h in range(4):
                t = stat.tile([64, D], cdt, tag=f"wp{h}")
                nc.sync.dma_start(t[:], wp_d[h * 64:(h + 1) * 64, :])
                wp.append(t)
        bqk = stat.tile([128, 4], f32, tag="bqk")
        nc.sync.dma_start(bqk[:], bqk_d[:])
        tri = stat.tile([128, 128], adt, tag="tri")
        nc.sync.dma_start(tri[:], tri_d[:])
        ones1 = stat.tile([1, 64], f32, tag="ones1")
        nc.vector.memset(ones1[:], 1.0)

        qkT = [stat.tile([128, S], adt, tag=f"qkT{m}", name=f"qkT{m}")
               for m in range(4)]
        vaug = [stat.tile([128, 260], adt, tag=f"vaug{s}", name=f"vaug{s}")
                for s in range(NST)]
        if xpart:
            # per (qb, pair): [128, 512] — rows 0-63 head 2p, 64-127 head 2p+1
            ctxp = [[stat.tile([128, QB], cdt, tag=f"ctx{qb}_{p}",
                                name=f"ctx{qb}_{p}")
                     for p in range(2)] for qb in range(NQB)]
        else:
            # per (qb, head): [65, 512] (row 64 = denominator)
            ctxp = [[stat.tile([65, QB], cdt, tag=f"ctx{qb}_{h}",
                                name=f"ctx{qb}_{h}")
                     for h in range(4)] for qb in range(NQB)]

        big = mode != "mixed"  # f32 tensors -> tighter SBUF budget
        expp = pool("expT", (6 if not xpart else 8) if big else 12)
        recp = pool("recip", 2 if big else 6)
        outst = pool("outst", (1 if not xpart else 2) if big else 3)

        pm = ppool("pm", 4)
        pcx = ppool("pcx", 2)
        pbc = ppool("pbc", 2)

        # ---- stage B: qkT[m][:, nb*512:...] = (x @ w_qk)^T + bias ----
        for m in range(4):
            for nb in range(4):
                ps = pm.tile([128, 512], f32, tag="mm")
                for d in range(NDC):
                    nc.tensor.matmul(ps[:],
                                     lhsT=(wqk[d][:, m * 128:(m + 1) * 128]),
                                     rhs=(xt[d][:, nb * 512:(nb + 1) * 512]),
                                     start=(d == 0), stop=(d == NDC - 1))
                nc.any.tensor_scalar_add(qkT[m][:, nb * 512:(nb + 1) * 512],
                                         ps[:], bqk[:, m:m + 1])

        # ---- stage C: v natural [s, 256] + ones columns ----
        for st in range(NST):
            ps = pm.tile([128, 512], f32, tag="mm")
            for d in range(NDC):
                nc.tensor.matmul(ps[:, 0:256],
                                 lhsT=(xt[d][:, st * 128:(st + 1) * 128]),
                                 rhs=(wv[d][:]),
                                 start=(d == 0), stop=(d == NDC - 1))
            va = vaug[st]
            for h in range(NHC):
                nc.any.tensor_copy(va[:, h * 65:h * 65 + 64],
                                   ps[:, h * 64:(h + 1) * 64])
                nc.gpsimd.memset(va[:, h * 65 + 64:h * 65 + 65], 1.0)

        # ---- stage D: attention ----
        for qb in range(NQB):
            q0 = qb * QB
            for h in range(NHC):
                qt = qkT[h // 2]
                kt = qkT[2 + h // 2]
                rb = 64 * (h % 2)
                n_kc = (qb + 1) * 4
                exp_tiles = []
                for kc in range(n_kc):
                    r = kc - qb * 4  # >= 0 -> diagonal band tile
                    c0 = max(r, 0) * 128
                    ps = pm.tile([128, 512], f32, tag="mm")
                    nc.tensor.matmul(
                        ps[:, c0:QB],
                        lhsT=(kt[rb:rb + 64, kc * 128:(kc + 1) * 128]),
                        rhs=(qt[rb:rb + 64, q0 + c0:q0 + QB]),
                        start=True, stop=True)
                    et = expp.tile([128, 512], adt, tag="expT")
                    nc.scalar.activation(et[:, c0:QB], ps[:, c0:QB], EXP)
                    if r >= 0:
                        nc.vector.tensor_mul(et[:, c0:c0 + 128],
                                             et[:, c0:c0 + 128], tri[:])
                    exp_tiles.append((et, c0))
                pc = pcx.tile([65, 512], f32, tag="ctx")
                for kc, (et, c0) in enumerate(exp_tiles):
                    nc.tensor.matmul(
                        pc[:, c0:QB],
                        lhsT=(vaug[kc][:, h * 65:(h + 1) * 65]),
                        rhs=(et[:, c0:QB]),
                        start=(kc == 0), stop=(kc == n_kc - 1))
                # normalization pieces
                rt = recp.tile([1, 512], f32, tag="recip")
                nc.vector.reciprocal(rt[0:1, :], pc[64:65, :])
                if xpart:
                    if h % 2 == 0:
                        bc = pbc.tile([128, 512], f32, tag="bc")
                    nc.tensor.matmul(bc[rb:rb + 64, :], lhsT=ones1[0:1, 0:64],
                                     rhs=rt[0:1, :], start=True, stop=True)
                    nc.vector.tensor_copy(ctxp[qb][h // 2][rb:rb + 64, :],
                                          pc[0:64, :])
                    if h % 2 == 1:
                        p = h // 2
                        nc.vector.tensor_mul(ctxp[qb][p][:], ctxp[qb][p][:],
                                             bc[:])
                else:
                    bc = pbc.tile([128, 512], f32, tag="bc")
                    nc.tensor.matmul(bc[0:64, :], lhsT=ones1[0:1, 0:64],
                                     rhs=rt[0:1, :], start=True, stop=True)
                    nc.vector.tensor_copy(ctxp[qb][h][0:64, :], pc[0:64, :])
                    nc.vector.tensor_mul(ctxp[qb][h][0:64, :],
                                         ctxp[qb][h][0:64, :], bc[0:64, :])

        # ---- stage E: partial output projection ----
        for st in range(NST):
            qb = st // 4
            sc = (st % 4) * 128
            so = outst.tile([128, D], f32, tag="outst")
            for nb in range(2):
                pp = pm.tile([128, 512], f32, tag="mm")
                if xpart:
                    for p in range(2):
                        nc.tensor.matmul(
                            pp[:],
                            lhsT=(ctxp[qb][p][:, sc:sc + 128]),
                            rhs=(wp[p][:, nb * 512:(nb + 1) * 512]),
                            start=(p == 0), stop=(p == 1))
                else:
                    for h in range(4):
                        nc.tensor.matmul(
                            pp[:],
                            lhsT=(ctxp[qb][h][0:64, sc:sc + 128]),
                            rhs=(wp[h][:, nb * 512:(nb + 1) * 512]),
                            start=(h == 0), stop=(h == 3))
                nc.any.tensor_copy(so[:, nb * 512:(nb + 1) * 512], pp[:])
            nc.sync.dma_start(out_d[st * 128:(st + 1) * 128, :], so[:])

    nc.compile()
    return nc, np_adt


def _get_program(mode, xpart):
    key = (mode, xpart)
    if key not in _CACHE:
        _CACHE[key] = _build(mode, xpart)
    return _CACHE[key]


def make_in_maps(x, w_qkv, b_qkv, w_proj, mode=None):
    """Build the 8 per-core input maps from full inputs."""
    mode = mode or MODE
    import ml_dtypes
    bf16_np = np.dtype(ml_dtypes.bfloat16)
    f32_np = np.dtype(np.float32)
    # must match the dram dtype declarations in _build
    if mode == "mixed":
        pdt_np, adt_np, cdt_np = f32_np, bf16_np, bf16_np
    elif mode == "bf16":
        pdt_np, adt_np, cdt_np = bf16_np, bf16_np, bf16_np
    else:
        pdt_np, adt_np, cdt_np = f32_np, f32_np, f32_np
    x = np.asarray(x, np.float32)
    w_qkv = np.asarray(w_qkv, np.float32)
    b_qkv = np.asarray(b_qkv, np.float32)
    w_proj = np.asarray(w_proj, np.float32)
    scale = 1.0 / np.sqrt(HD)
    tri = np.triu(np.ones((128, 128), np.float32)).astype(adt_np)
    in_maps = []
    for c in range(N_CORES):
        b, hg = c // 4, c % 4
        hs = [hg * NHC + j for j in range(NHC)]
        wq = np.concatenate([w_qkv[:, h * HD:(h + 1) * HD] for h in hs], 1) * scale
        wk = np.concatenate([w_qkv[:, D + h * HD:D + (h + 1) * HD] for h in hs], 1)
        wqk = np.ascontiguousarray(np.concatenate([wq, wk], 1))
        bq = np.concatenate([b_qkv[h * HD:(h + 1) * HD] for h in hs]) * scale
        bk = np.concatenate([b_qkv[D + h * HD:D + (h + 1) * HD] for h in hs])
        bqk = np.ascontiguousarray(
            np.concatenate([bq, bk]).reshape(4, 128).T)
        wv = np.ascontiguousarray(
            w_qkv[:, 2 * D + hg * 256:2 * D + (hg + 1) * 256])
        wp = np.ascontiguousarray(w_proj[hg * 256:(hg + 1) * 256, :])
        in_maps.append({
            "xt": np.ascontiguousarray(x[b].T).astype(pdt_np),
            "wqk": wqk.astype(pdt_np),
            "wv": wv.astype(pdt_np),
            "wp": wp.astype(cdt_np),
            "bqk": bqk,
            "tri": tri,
        })
    return in_maps


def assemble_output(results, b_qkv, b_proj, w_proj):
    """Sum per-core partials; add v-bias and proj-bias contributions."""
    out = np.zeros((B, S, D), np.float32)
    for c in range(N_CORES):
        out[c // 4] += results[c]["out"]
    bv = np.asarray(b_qkv, np.float32)[2 * D:]
    out += (bv @ np.asarray(w_proj, np.float32) +
            np.asarray(b_proj, np.float32))[None, None, :]
    return out


def kernel(x, w_qkv, b_qkv, w_proj, b_proj):
    from concourse.bass_utils import run_bass_kernel_spmd
    nc, _ = _get_program(MODE, XPART)
    in_maps = make_in_maps(x, w_qkv, b_qkv, w_proj)
    res = run_bass_kernel_spmd(nc, in_maps, list(range(N_CORES)))
    return assemble_output(res.results, b_qkv, b_proj, w_proj)
